# revision 1
# baseline (speedup 1.0000x reference)
"""AttentionBlock SPMD kernel for 8 TRN2 NeuronCores.

Math (matching the reference):
  qkv = x @ W_qkv + b_qkv -> q,k,v per (b, h)
  scores = q k^T / sqrt(64) + bias[h];  attn = softmax(scores)
  out = (attn @ v  concat heads) @ W_proj + b_proj

Sharding: 48 units of (head h, query block qb of 512 rows), batch kept
together per unit so each bias slice is read once fleet-wide. Core c owns
head A=c (4 blocks) and head B=8+c//2 (2 blocks; odd cores work in a
column-permuted coordinate system so the program is SPMD-uniform).

On-device layout: S is computed transposed (keys on partitions, queries on
free dim), so exp(S^T) is directly the stationary operand of the PV matmul
and the softmax denominator falls out of an extra ones-column in v.
Matmuls run as float32r (full PE speed, ~1e-4 rel precision). The bias add
is split between DVE (12/16 chunks) and a PE identity-matmul accumulation
(4/16). Per-(b) and per-(qb,b) tile granularity lets qkv, attention, and
projection phases overlap. Each core emits a partial
out^T = sum_{h in core} W_proj[h].T @ attn^T[h]; the host sums partials,
adds b_proj and untransposes.
"""

import numpy as np

B, N, D, H, HD = 4, 2048, 768, 12, 64
R = B * N                # 8192 flattened rows
P = 128                  # partitions
FD = 512                 # free-dim tile (query block)
NKD = D // P             # 6 contraction chunks over D
NRT = R // FD            # 16 row tiles
NKC = N // P             # 16 key chunks per batch
NCORES = 8
SCALE = 1.0 / np.sqrt(HD)

# phase-2 unit order: B-head units early so projection of the covered
# query blocks can start mid-phase.  (slot, qb): slot 1 = head B.
ULIST = [(1, 0), (0, 0), (1, 1), (0, 1), (0, 2), (0, 3)]
# after unit index ui completes, these query blocks are fully computed
PROJ_AFTER = {0: [], 1: [0], 2: [], 3: [1], 4: [2], 5: [3]}

_NC = None               # compiled module cache
TRACE = False
LAST_PROFILE = None


def _build_module(reps=1):
    import concourse.bacc as bacc
    import concourse.tile as tile
    from concourse import mybir
    from concourse.masks import make_identity

    f32 = mybir.dt.float32
    f32r = mybir.dt.float32r

    nc = bacc.Bacc("TRN2", target_bir_lowering=False, debug=False,
                   num_devices=NCORES)

    xT_d = nc.dram_tensor("xT", [D, R], f32r, kind="ExternalInput").ap()
    wq_d = nc.dram_tensor("wq", [D, P], f32r, kind="ExternalInput").ap()
    wk_d = nc.dram_tensor("wk", [D, P], f32r, kind="ExternalInput").ap()
    wv_d = nc.dram_tensor("wv", [D, P], f32r, kind="ExternalInput").ap()
    bqkv_d = nc.dram_tensor("bqkv", [P, 3], f32, kind="ExternalInput").ap()
    biasT_d = nc.dram_tensor("biasT", [6, N, FD], f32r, kind="ExternalInput").ap()
    identr_d = nc.dram_tensor("identr", [P, P], f32r, kind="ExternalInput").ap()
    wproj_d = nc.dram_tensor("wproj", [P, D], f32r, kind="ExternalInput").ap()
    outT_d = nc.dram_tensor("outT", [D, R], f32, kind="ExternalOutput").ap()

    xT_r = xT_d.rearrange("(kc p) r -> p kc r", p=P)          # (128, 6, 8192)

    with tile.TileContext(nc) as tc:
        with (
            tc.tile_pool(name="const", bufs=1) as const,
            tc.tile_pool(name="persist", bufs=1) as persist,
            tc.tile_pool(name="psum_mm", bufs=2, space="PSUM") as psmm,
            tc.tile_pool(name="psum_pj", bufs=1, space="PSUM") as pspj,
            tc.tile_pool(name="psum_av", bufs=4, space="PSUM") as psav,
            tc.tile_pool(name="psum_tr", bufs=1, space="PSUM") as pstr,
            tc.tile_pool(name="xt", bufs=2) as xtp,
            tc.tile_pool(name="wk1", bufs=1) as wk1,
            tc.tile_pool(name="bias", bufs=3) as biasp,
            tc.tile_pool(name="wk2", bufs=3) as wk2,
            tc.tile_pool(name="small", bufs=1) as small,
            tc.tile_pool(name="wk3", bufs=2) as wk3,
        ):
            # ---- constants ----
            wq_t = const.tile([P, NKD, P], f32r)
            wk_t = const.tile([P, NKD, P], f32r)
            wv_t = const.tile([P, NKD, P], f32r)
            for w_t, w_d in ((wq_t, wq_d), (wk_t, wk_d), (wv_t, wv_d)):
                nc.sync.dma_start(out=w_t, in_=w_d.rearrange("(kc p) m -> p kc m", p=P))
            bqkv_t = const.tile([P, 3], f32)
            nc.sync.dma_start(out=bqkv_t, in_=bqkv_d)
            wproj_t = const.tile([P, D], f32r)
            nc.sync.dma_start(out=wproj_t, in_=wproj_d)
            ident = const.tile([P, P], f32)
            make_identity(nc, ident)
            identr = const.tile([P, P], f32r)
            nc.sync.dma_start(out=identr, in_=identr_d)
            ones64 = const.tile([1, 64], f32)
            nc.vector.memset(ones64, 1.0)
            ones128 = const.tile([P, 2 * NKC], f32)
            nc.vector.memset(ones128, 1.0)

            # ---- persistent buffers (per-b granularity for overlap) ----
            # partition halves: rows 0-63 = head A, 64-127 = head B
            qTb = [persist.tile([P, N], f32r, name=f"qT{b}") for b in range(B)]
            kTb = [persist.tile([P, N], f32r, name=f"kT{b}") for b in range(B)]
            # v keys-major: [key_in_chunk, slot, kc, hd(64)+ones(1)]
            vb = [persist.tile([P, 2, NKC, HD + 1], f32r, name=f"v{b}")
                  for b in range(B)]
            # attn^T per (qb, b): rows = 2 head slots
            atb = [[persist.tile([P, FD], f32r, name=f"at{qb}_{b}")
                    for b in range(B)] for qb in range(4)]
            for b in range(B):
                nc.vector.tensor_copy(
                    out=vb[b][:, :, :, HD:HD + 1],
                    in_=ones128.rearrange("p (a c) -> p a c", a=2)[:, :, :, None])

            for _rep in range(reps):
                # ---- phase 1: qkv projection (+ v transpose) ----
                    for rt in range(NRT):
                        b_i, qb = rt // 4, rt % 4
                        cols = slice(rt * FD, (rt + 1) * FD)
                        qcols = slice(qb * FD, (qb + 1) * FD)
                        xt = xtp.tile([P, NKD, FD], f32r, tag="xt")
                        eng = nc.sync if rt % 2 == 0 else nc.gpsimd
                        eng.dma_start(out=xt, in_=xT_r[:, :, cols])
                        for g, (w_t, bcol) in enumerate(
                                ((wq_t, 0), (wk_t, 1), (wv_t, 2))):
                            ps = psmm.tile([P, FD], f32, tag="mm")
                            for kc in range(NKD):
                                nc.tensor.matmul(ps, w_t[:, kc, :], xt[:, kc, :],
                                                 start=(kc == 0),
                                                 stop=(kc == NKD - 1))
                            Ident = mybir.ActivationFunctionType.Identity
                            if g == 0:
                                nc.scalar.activation(
                                    qTb[b_i][0:64, qcols], ps[0:64, :], Ident,
                                    bias=bqkv_t[0:64, bcol:bcol + 1])
                                if qb < 2:
                                    nc.scalar.activation(
                                        qTb[b_i][64:128, qcols], ps[64:128, :],
                                        Ident, bias=bqkv_t[64:128, bcol:bcol + 1])
                            elif g == 1:
                                nc.vector.tensor_scalar_add(
                                    kTb[b_i][:, qcols], ps,
                                    bqkv_t[:, bcol:bcol + 1])
                            else:
                                vt_sb = wk1.tile([P, FD], f32, tag="vt")
                                nc.scalar.activation(
                                    vt_sb, ps, Ident,
                                    bias=bqkv_t[:, bcol:bcol + 1])
                                for j in range(4):
                                    kc = qb * 4 + j
                                    tp = pstr.tile([P, P], f32, tag="tr")
                                    nc.tensor.transpose(
                                        tp, vt_sb[:, j * P:(j + 1) * P], ident)
                                    nc.vector.tensor_copy(
                                        out=vb[b_i][:, :, kc, 0:HD],
                                        in_=tp.rearrange("p (s hd) -> p s hd", s=2))

                # ---- phase 2: attention ----
                    for ui, (slot, qb) in enumerate(ULIST):
                        pb = slot * 64
                        qcols = slice(qb * FD, (qb + 1) * FD)
                        av = [psav.tile([HD + 1, FD], f32, tag="av",
                                        name=f"av_u{ui}b{bb}") for bb in range(B)]
                        for half in range(8):
                            bt = biasp.tile([P, 2, FD], f32r, tag="bt")
                            nc.gpsimd.dma_start(
                                out=bt,
                                in_=biasT_d[ui].rearrange("(kc p) q -> p kc q", p=P)[
                                    :, half * 2:(half + 1) * 2, :])
                            for b_i in range(B):
                                for j in range(2):
                                    kc = half * 2 + j
                                    pe_bias = half >= 6   # kc 12-15 via PE
                                    kcols = slice(kc * P, (kc + 1) * P)
                                    sps = psmm.tile([P, FD], f32, tag="mm")
                                    nc.tensor.matmul(
                                        sps, kTb[b_i][pb:pb + 64, kcols],
                                        qTb[b_i][pb:pb + 64, qcols],
                                        start=True, stop=not pe_bias)
                                    if pe_bias:
                                        nc.tensor.matmul(
                                            sps, identr, bt[:, j, :],
                                            start=False, stop=True)
                                        src_e = sps
                                    else:
                                        ex = wk2.tile([P, FD], f32, tag="ex")
                                        nc.vector.tensor_add(ex, sps, bt[:, j, :])
                                        src_e = ex
                                    eq = wk2.tile([P, FD], f32r, tag="eq")
                                    nc.scalar.activation(
                                        eq, src_e, mybir.ActivationFunctionType.Exp)
                                    nc.tensor.matmul(
                                        av[b_i], vb[b_i][:, slot, kc, :],
                                        eq, start=(kc == 0), stop=(kc == NKC - 1))
                        for b_i in range(B):
                            rd = small.tile([1, FD], f32, tag="rd")
                            sc = small.tile([1, FD], f32, tag="sc")
                            dn = small.tile([1, FD], f32, tag="dn")
                            rb = small.tile([64, FD], f32, tag="rb")
                            nc.scalar.copy(dn, av[b_i][HD:HD + 1, :])
                            nc.vector.reciprocal_approx_accurate(
                                out=rd, in_=dn, scratch=sc)
                            # broadcast recip row to 64 partitions via K=1 matmul
                            rb_ps = pstr.tile([64, FD], f32, tag="tr")
                            nc.tensor.matmul(rb_ps, ones64, rd,
                                             start=True, stop=True)
                            nc.scalar.copy(rb, rb_ps)
                            nc.vector.tensor_mul(
                                atb[qb][b_i][pb:pb + 64, :], av[b_i][0:HD, :], rb)
                        # projection for every query block whose units are done
                        for pqb in PROJ_AFTER[ui]:
                            covered = pqb < 2
                            for b_i in range(B):
                                cols = slice(b_i * N + pqb * FD,
                                             b_i * N + (pqb + 1) * FD)
                                for mc in range(NKD):
                                    mslice = slice(mc * P, (mc + 1) * P)
                                    ps = pspj.tile([P, FD], f32, tag="proj")
                                    if covered:
                                        nc.tensor.matmul(ps, wproj_t[:, mslice],
                                                         atb[pqb][b_i],
                                                         start=True, stop=True)
                                    else:
                                        nc.tensor.matmul(ps, wproj_t[0:64, mslice],
                                                         atb[pqb][b_i][0:64, :],
                                                         start=True, stop=True)
                                    ot = wk3.tile([P, FD], f32, tag="ot")
                                    if mc % 2 == 0:
                                        nc.vector.tensor_copy(ot, ps)
                                    else:
                                        nc.scalar.copy(ot, ps)
                                    nc.sync.dma_start(out=outT_d[mslice, cols],
                                                      in_=ot)



    nc.compile()
    return nc


def _get_module():
    global _NC
    if _NC is None:
        _NC = _build_module()
    return _NC


def _host_pack(x, bias, W_qkv, b_qkv, W_proj):
    """Build the 8 per-core input maps."""
    x = np.asarray(x, dtype=np.float32)
    bias = np.asarray(bias, dtype=np.float32)
    W_qkv = np.asarray(W_qkv, dtype=np.float32)
    b_qkv = np.asarray(b_qkv, dtype=np.float32)
    W_proj = np.asarray(W_proj, dtype=np.float32)

    xT = np.ascontiguousarray(x.reshape(R, D).T)              # (768, 8192)
    # odd cores: n -> (n + 1024) % 2048 within each batch
    xT_odd = np.ascontiguousarray(
        xT.reshape(D, B, 2, N // 2)[:, :, ::-1, :].reshape(D, R))
    identr = np.eye(P, dtype=np.float32)

    in_maps = []
    for c in range(NCORES):
        hA, hB, halfB = c, 8 + c // 2, c % 2
        odd = halfB == 1

        def wcols(which, h):
            return W_qkv[:, which * D + h * HD: which * D + (h + 1) * HD]

        wq = np.concatenate([wcols(0, hA), wcols(0, hB)], axis=1) * SCALE
        wk = np.concatenate([wcols(1, hA), wcols(1, hB)], axis=1)
        wv = np.concatenate([wcols(2, hA), wcols(2, hB)], axis=1)
        bq = np.concatenate([b_qkv[hA * HD:(hA + 1) * HD],
                             b_qkv[hB * HD:(hB + 1) * HD]]) * SCALE
        bk = np.concatenate([b_qkv[D + hA * HD: D + (hA + 1) * HD],
                             b_qkv[D + hB * HD: D + (hB + 1) * HD]])
        bv = np.concatenate([b_qkv[2 * D + hA * HD: 2 * D + (hA + 1) * HD],
                             b_qkv[2 * D + hB * HD: 2 * D + (hB + 1) * HD]])
        bqkv = np.stack([bq, bk, bv], axis=1)                  # (128, 3)

        def head_bias(h):
            hb = bias[0, h]                                    # (q, k) true coords
            if odd:
                hb = np.roll(np.roll(hb, -N // 2, axis=0), -N // 2, axis=1)
            return hb

        bA, bB = head_bias(hA), head_bias(hB)
        biasT = np.empty((6, N, FD), dtype=np.float32)
        for ui, (slot, qb) in enumerate(ULIST):
            h_m = bB if slot == 1 else bA
            biasT[ui] = h_m[qb * FD:(qb + 1) * FD, :].T

        wproj = np.concatenate([W_proj[hA * HD:(hA + 1) * HD, :],
                                W_proj[hB * HD:(hB + 1) * HD, :]], axis=0)

        in_maps.append({
            "xT": xT_odd if odd else xT,
            "identr": identr,
            "wq": np.ascontiguousarray(wq), "wk": np.ascontiguousarray(wk),
            "wv": np.ascontiguousarray(wv),
            "bqkv": np.ascontiguousarray(bqkv),
            "biasT": biasT,
            "wproj": np.ascontiguousarray(wproj),
        })
    return in_maps


def kernel(x, bias, W_qkv, b_qkv, W_proj, b_proj):
    global LAST_PROFILE
    from concourse.bass_utils import run_bass_kernel_spmd

    nc = _get_module()
    in_maps = _host_pack(x, bias, W_qkv, b_qkv, W_proj)
    res = run_bass_kernel_spmd(nc, in_maps, list(range(NCORES)),
                               trace=TRACE)
    LAST_PROFILE = res
    outT = np.zeros((D, R), dtype=np.float64)
    for c in range(NCORES):
        part = np.asarray(res.results[c]["outT"], dtype=np.float64)
        if c % 2 == 1:  # undo column permutation (involution)
            part = part.reshape(D, B, 2, N // 2)[:, :, ::-1, :].reshape(D, R)
        outT += part
    out = outT.T.astype(np.float32) + np.asarray(b_proj, dtype=np.float32)
    return out.reshape(B, N, D)

